# revision 36
# baseline (speedup 1.0000x reference)
"""AttentionSSA Trainium2 Bass kernel (v2).

Computation (per batch b):
  qkv = x @ qkv_w + qkv_b ; split into per-head q,k,v
  S = (q @ k^T) * scale
  attn = softmax(w)[0] * softmax(S) + softmax(w)[1] * relu(S)^2
  out = (attn @ v) reassembled, @ proj_w + proj_b

Sharding: data-parallel over batch B=16 across 8 NeuronCores (2 batches/core).
Each core computes its slice fully independently (no collectives).

v2 design notes (vs v1 baseline at 742us):
 - x is pre-transposed on the HOST -> xT [768, 1280]; the whole P1 PE
   transpose phase is gone.
 - All activations forced into ONE ACT table set (natural_log_exp_and_others)
   by patching bacc's get_activation_tables: kills 49 ACT_TABLE_LOADs.
 - Bias adds fused into PSUM->SBUF evictions (tensor_scalar with per-partition
   bias column for q/k; scalar_tensor_tensor with a host-replicated bias tile
   for v and proj) -> all K=1 ones-row bias matmuls gone.
 - P2(b1) and P4(b0) matmul chunks are interleaved into the P3 head loop so
   the PE never idles long enough for HAM to re-throttle to 1.2 GHz.
 - relu^2 branch moved off ACT: relu on DVE (tensor_scalar mult+max),
   square on gpsimd (tensor_tensor) - ACT only does exp + ln/exp recip chain.
 - PSUM: one shared [128,768] pool (bufs=2) for qkv/ST/proj outputs + av0/av1.
"""
import math
from contextlib import ExitStack

import numpy as np

import concourse.bacc as bacc
import concourse.bass as bass
import concourse.mybir as mybir
import concourse.tile as tile
from concourse.bass_utils import run_bass_kernel_spmd

F32 = mybir.dt.float32
F16 = mybir.dt.float16
AF = mybir.ActivationFunctionType
ALU = mybir.AluOpType

NCORES = 8
B, N, D, H, DH = 16, 640, 768, 12, 64
BPC = B // NCORES          # batches per core
TOK = BPC * N              # tokens per core (1280)
SCALE = DH ** -0.5
KT = 5                     # 640/128 token tiles per batch
FT = 6                     # 768/128 dim tiles
VW = DH + 1                # 65: per-head v block [64 feats | ones col]

# ---- engine assignment tunables ----
QK_EV = "scalar"           # q/k eviction (+bias col): "scalar" ACT / "vector" DVE
RELU_ENG = ["vector"] * KT   # relu stage per kt: "vector" or "scalar"
SQ_ENG = ["gpsimd"] * KT     # square stage per kt: "gpsimd" or "vector"

_ACT_TABLES_PATCHED = False


def _patch_act_tables():
    """Force every activation into one table set that covers Exp/Ln/Relu/
    Copy/Identity, so no ACT_TABLE_LOAD thrash at runtime. Positions of the
    sets are preserved (ids are positional), other sets are just emptied."""
    global _ACT_TABLES_PATCHED
    if _ACT_TABLES_PATCHED:
        return
    _ACT_TABLES_PATCHED = True
    orig = bacc.get_activation_tables
    need = {AF.Exp, AF.Ln, AF.Relu, AF.Copy, AF.Identity, AF.Square}

    def patched(arch):
        t = orig(arch)
        target = None
        for name, fns in t.items():
            if need <= fns:
                target = name
                break
        if target is None:
            return t
        return {name: (fns if name == target else set())
                for name, fns in t.items()}

    bacc.get_activation_tables = patched


def build_nc():
    _patch_act_tables()
    nc = bacc.Bacc("TRN2", target_bir_lowering=False, debug=False)

    xt_d = nc.dram_tensor("xT", [D, TOK], F16, kind="ExternalInput")
    qkvw_d = nc.dram_tensor("qkv_w", [D, 3 * D], F16, kind="ExternalInput")
    projw_d = nc.dram_tensor("proj_w", [D, D], F16, kind="ExternalInput")
    auxb_d = nc.dram_tensor("auxb", [128, 13], F32, kind="ExternalInput")
    vbb_d = nc.dram_tensor("vbb", [128, D], F32, kind="ExternalInput")
    pbb_d = nc.dram_tensor("pbb", [128, D], F32, kind="ExternalInput")
    out_d = nc.dram_tensor("out", [TOK, D], F32, kind="ExternalOutput")
    drec_d = nc.dram_tensor("drec_scratch", [3, N], F16, kind="Internal")

    with tile.TileContext(nc) as tc, ExitStack() as ctx:
        perm = ctx.enter_context(tc.tile_pool(name="perm", bufs=1))
        AXB = perm.tile([128, 13], F32, tag="auxb")
        VBB = perm.tile([128, D], F32, tag="vbb")
        PBB = perm.tile([128, D], F32, tag="pbb")
        XT = perm.tile([128, FT * TOK], F16, tag="xt")
        WQ = perm.tile([128, FT * 3 * D], F16, tag="wq")
        PW = perm.tile([128, FT * D], F16, tag="pw")
        QK = perm.tile([128, BPC * 12 * N], F16, tag="qk")
        VR = perm.tile([128, BPC * KT * H * VW], F16, tag="vr")
        YT = perm.tile([128, BPC * FT * N], F16, tag="yt")

        nc.sync.dma_start(AXB[:], auxb_d[:])
        nc.sync.dma_start(VBB[:], vbb_d[:])
        nc.sync.dma_start(PBB[:], pbb_d[:])
        for k in range(FT):
            nc.sync.dma_start(XT[:, k * TOK:(k + 1) * TOK],
                              xt_d[k * 128:(k + 1) * 128, :])
            nc.sync.dma_start(WQ[:, k * 3 * D:(k + 1) * 3 * D],
                              qkvw_d[k * 128:(k + 1) * 128, :])
            nc.sync.dma_start(PW[:, k * D:(k + 1) * D],
                              projw_d[k * 128:(k + 1) * 128, :])

        p1sc = AXB[:, 12:13]

        def qk_col(b, f, c):
            return (b * 12 + f) * N + c

        def v_col(b, kt, c):
            return (b * KT + kt) * H * VW + c

        def yt_col(b, pi, c):
            return (b * FT + pi) * N + c

        # ones columns of VR (col 64 of each per-head 65 block), once
        vones = VR[:].rearrange("p (g c) -> p g c", c=VW)[:, :, DH:DH + 1]
        nc.vector.memset(vones, 1.0)

        outs = ctx.enter_context(tc.tile_pool(name="pouts", bufs=3))
        pp0 = ctx.enter_context(tc.tile_pool(name="pp0", bufs=16))
        ppr = ctx.enter_context(tc.tile_pool(name="ppr", bufs=6))
        pp1 = ctx.enter_context(tc.tile_pool(name="pp1", bufs=16))
        psm = ctx.enter_context(tc.tile_pool(name="psm", bufs=3))
        pbc = ctx.enter_context(tc.tile_pool(name="pbc", bufs=3))
        ps_main = ctx.enter_context(
            tc.tile_pool(name="psmain", bufs=2, space="PSUM"))
        ps_a0 = ctx.enter_context(
            tc.tile_pool(name="psa0", bufs=1, space="PSUM"))
        ps_a1 = ctx.enter_context(
            tc.tile_pool(name="psa1", bufs=1, space="PSUM"))

        # ---------------- emission helpers ----------------
        def emit_qk(b, f):
            """q (f<6) / k (f>=6) feature tile f -> QK[:, qk_col(b,f,:)]."""
            fcol = f * 128 if f < 6 else 768 + (f - 6) * 128
            qp = ps_main.tile([128, D], F32, tag="mm")
            for off, wd in ((0, 512), (512, 128)):
                for k in range(FT):
                    nc.tensor.matmul(
                        qp[:, off:off + wd],
                        WQ[:, k * 3 * D + fcol:k * 3 * D + fcol + 128],
                        XT[:, k * TOK + b * N + off:k * TOK + b * N + off + wd],
                        start=(k == 0), stop=(k == FT - 1))
            dst = QK[:, qk_col(b, f, 0):qk_col(b, f, N)]
            if QK_EV == "scalar":
                nc.scalar.activation(dst, qp[:, 0:N], AF.Identity,
                                     bias=AXB[:, f:f + 1], scale=1.0)
            else:
                nc.vector.tensor_scalar(dst, qp[:, 0:N], AXB[:, f:f + 1],
                                        None, ALU.add)

        def emit_v(b, t):
            """v token tile t -> VR strided per-head blocks (w0 pre-folded)."""
            vp = ps_main.tile([128, D], F32, tag="mm")
            for off, wd in ((0, 512), (512, 256)):
                for k in range(FT):
                    nc.tensor.matmul(
                        vp[:, off:off + wd],
                        XT[:, k * TOK + b * N + t * 128:
                              k * TOK + b * N + (t + 1) * 128],
                        WQ[:, k * 3 * D + 1536 + off:
                              k * 3 * D + 1536 + off + wd],
                        start=(k == 0), stop=(k == FT - 1))
            vdst = VR[:, v_col(b, t, 0):v_col(b, t, H * VW)] \
                .rearrange("p (h c) -> p h c", h=H)[:, :, 0:DH]
            nc.vector.scalar_tensor_tensor(
                vdst, vp[:].rearrange("p (h c) -> p h c", h=H),
                1.0, VBB[:].rearrange("p (h c) -> p h c", h=H),
                ALU.mult, ALU.add)

        def emit_p4_early(b, t, fhi=FT):
            """proj accumulation over f<fhi for token tile t (no evict)."""
            op = ps_main.tile([128, D], F32, tag="mm")
            for off, wd in ((0, 512), (512, 256)):
                for f in range(fhi):
                    nc.tensor.matmul(
                        op[:, off:off + wd],
                        YT[:, yt_col(b, f, t * 128):yt_col(b, f, (t + 1) * 128)],
                        PW[:, f * D + off:f * D + off + wd],
                        start=(f == 0), stop=(f == FT - 1),
                        skip_group_check=True)
            return op

        def emit_p4_late(b, t, op, flo=FT):
            """remaining proj accumulation (f>=flo) + evict + DMA out."""
            for off, wd in ((0, 512), (512, 256)):
                for f in range(flo, FT):
                    nc.tensor.matmul(
                        op[:, off:off + wd],
                        YT[:, yt_col(b, f, t * 128):yt_col(b, f, (t + 1) * 128)],
                        PW[:, f * D + off:f * D + off + wd],
                        start=False, stop=(f == FT - 1),
                        skip_group_check=True)
            ot = outs.tile([128, D], F32, tag="outs")
            nc.vector.scalar_tensor_tensor(ot[:], op[:], 1.0, PBB[:],
                                           ALU.mult, ALU.add)
            g = b * KT + t
            nc.sync.dma_start(out_d[g * 128:(g + 1) * 128, :], ot[:])

        def emit_p4(b, t):
            emit_p4_late(b, t, emit_p4_early(b, t))

        def emit_st(b, h, kt, p0s, p1s):
            pi, po = h // 2, 64 * (h % 2)
            st = ps_main.tile([128, N], F32, tag="mm")
            for off, wd in ((0, 512), (512, 128)):
                nc.tensor.matmul(
                    st[:, off:off + wd],
                    QK[po:po + 64, qk_col(b, 6 + pi, kt * 128):
                                   qk_col(b, 6 + pi, (kt + 1) * 128)],
                    QK[po:po + 64, qk_col(b, pi, off):
                                   qk_col(b, pi, off + wd)],
                    start=True, stop=True)
            p0 = pp0.tile([128, N], F16, tag="p0")
            nc.scalar.activation(p0[:], st[:], AF.Exp,
                                 bias=0.0, scale=SCALE)
            r = ppr.tile([128, N], F16, tag="r")
            if RELU_ENG[kt] == "scalar":
                nc.scalar.activation(r[:], st[:], AF.Relu,
                                     bias=0.0, scale=p1sc)
            else:
                nc.vector.tensor_scalar(r[:], st[:], p1sc, 0.0,
                                        ALU.mult, ALU.max)
            p1 = pp1.tile([128, N], F16, tag="p1")
            if SQ_ENG[kt] == "gpsimd":
                nc.gpsimd.tensor_tensor(p1[:], r[:], r[:], ALU.mult)
            else:
                nc.vector.tensor_tensor(p1[:], r[:], r[:], ALU.mult)
            p0s.append(p0)
            p1s.append(p1)

        av_seq = [0]

        def emit_av0(b, h, p0s):
            av0 = ps_a0.tile([65, N], F32, tag="av0")
            for kt in range(KT):
                for off, wd in ((0, 512), (512, 128)):
                    sl = slice(off, off + wd)
                    nc.tensor.matmul(
                        av0[0:65, sl],
                        VR[:, v_col(b, kt, h * VW):v_col(b, kt, h * VW + VW)],
                        p0s[kt][:, sl],
                        start=(kt == 0), stop=(kt == KT - 1))
            # start the reciprocal chain right away - it only needs av0
            dln = psm.tile([1, N], F32, tag="dln")
            nc.scalar.activation(dln[:], av0[64:65, :], AF.Ln,
                                 bias=0.0, scale=1.0)
            drec = psm.tile([1, N], F16, tag="drec")
            nc.scalar.activation(drec[:], dln[:], AF.Exp,
                                 bias=0.0, scale=-1.0)
            # partition-broadcast drec via DMA round-trip through DRAM (SBUF
            # APs reject step-0 partitions, DRAM APs allow them): keeps
            # gpsimd entirely in ucode library 0 (tensor_tensor squares) -
            # mixing in partition_broadcast (library 1) forced a 35KB ucode
            # reload around every head's broadcast (~7us on the critical path).
            slot = av_seq[0] % 3
            av_seq[0] += 1
            nc.sync.dma_start(drec_d[slot:slot + 1, :], drec[:])
            bc = pbc.tile([64, N], F16, tag="bc")
            dsl = drec_d[slot:slot + 1, :]
            src = bass.AP(tensor=dsl.tensor, offset=dsl.offset,
                          ap=[[0, 64]] + list(dsl.ap[1:]))
            nc.sync.dma_start(bc[:], src)
            return av0, bc

        def emit_av1c(b, h, pi, po, p1s, av0, bc):
            av1 = ps_a1.tile([64, N], F32, tag="av1")
            for kt in range(KT):
                for off, wd in ((0, 512), (512, 128)):
                    sl = slice(off, off + wd)
                    nc.tensor.matmul(
                        av1[0:64, sl],
                        VR[:, v_col(b, kt, h * VW):v_col(b, kt, h * VW + DH)],
                        p1s[kt][:, sl],
                        start=(kt == 0), stop=(kt == KT - 1))
            # combine: YT = av0[0:64] * (1/denom) + av1
            tmp = psm.tile([64, N], F32, tag="tmp")
            nc.vector.tensor_tensor(tmp[:], av0[0:64, :], bc[:], ALU.mult)
            nc.vector.tensor_tensor(
                YT[po:po + 64, yt_col(b, pi, 0):yt_col(b, pi, N)],
                tmp[:], av1[0:64, :], ALU.add)

        # ---------------- main schedule ----------------
        # HAM warm-up: ~4us of tiny matmuls while the weight DMAs land, so
        # the PE clock is already at 2.4GHz when P2 starts (the activity
        # monitor needs ~3.4us of sustained PE work to unthrottle).
        wt = ps_main.tile([128, D], F32, tag="mm")
        for _ in range(150):
            nc.tensor.matmul(wt[0:13, 0:1], AXB[:, 0:13], AXB[:, 0:1],
                             start=True, stop=True)

        # minimal prologue: v(b0) (needed by the first AV at iteration 2) and
        # the first q/k feature pair; everything else streams into the loop.
        for t in range(KT):
            emit_v(0, t)
        emit_qk(0, 0)
        emit_qk(0, 6)

        # extra PE chunks interleaved into the P3 head loop. head (b, 2p) and
        # (b, 2p+1) read qk feature tiles p and 6+p, so pair p must be
        # emitted before iteration b*12 + 2p; P4(b0) needs all of YT(b0),
        # complete after the AV of head 11 (emitted during iteration 13).
        # 2 chunks/iter early + 1/iter for i=12..17 keeps the ps_main ring
        # pressure even; the last two P4(b0) chunks are held back to fill
        # the PE while the final heads' combine chains drain.
        extras = []
        for p in range(1, 6):
            extras += [lambda f=p: emit_qk(0, f), lambda f=p: emit_qk(0, 6 + f)]
        extras += [lambda t=t: emit_v(1, t) for t in range(KT)]
        for p in range(6):
            extras += [lambda f=p: emit_qk(1, f), lambda f=p: emit_qk(1, 6 + f)]
        extras_b = [lambda t=t: emit_p4(0, t) for t in range(KT)]
        NTAKE = [2] * 12 + [1] * 12

        # software pipeline, 2 heads deep. The AV matmul groups of head i-2
        # are interleaved INTO head i's ST tile sequence, so the PE has dense
        # queued work while the elementwise engines consume head i's score
        # tiles (st tile j+2 can only start once exp+relu of tile j freed its
        # PSUM buffer - without filler the PE idles at that cadence).
        bh_list = [(b, h) for b in range(BPC) for h in range(H)]
        pend = []
        for i, (b, h) in enumerate(bh_list):
            prev = pend.pop(0) if len(pend) >= 2 else None
            p0s, p1s = [], []
            emit_st(b, h, 0, p0s, p1s)
            emit_st(b, h, 1, p0s, p1s)
            if prev is not None:
                prev_av0 = emit_av0(prev[0], prev[1], prev[4])
            emit_st(b, h, 2, p0s, p1s)
            if prev is not None:
                emit_av1c(prev[0], prev[1], prev[2], prev[3], prev[5],
                          *prev_av0)
            emit_st(b, h, 3, p0s, p1s)
            for _ in range(NTAKE[i]):
                if extras:
                    extras.pop(0)()
                elif extras_b and i >= 14 and len(extras_b) > 2:
                    extras_b.pop(0)()
            emit_st(b, h, 4, p0s, p1s)
            pend.append((b, h, h // 2, 64 * (h % 2), p0s, p1s))
        # tail: fill the PE while the last two heads' combine chains drain.
        # P4(b1) f=0..4 only needs YT(b1) columns written by heads <=9 (done
        # during the loop), so those accumulation groups interleave into the
        # drain gaps; only the f=5 matmuls + evictions wait for the very
        # last YT write. (each early op tile must have its late emitted
        # before the early two-allocations-later, or the ps_main ring
        # deadlocks.)
        eops = []
        while pend:
            b2, h2, pi2, po2, p0s2, p1s2 = pend.pop(0)
            av0, bc = emit_av0(b2, h2, p0s2)
            if extras_b:
                extras_b.pop(0)()
            emit_av1c(b2, h2, pi2, po2, p1s2, av0, bc)
            if not eops:
                eops.append(emit_p4_early(1, 0, fhi=FT - 1))
        while extras_b:
            extras_b.pop(0)()
        for t in range(KT):
            emit_p4_late(1, t, eops[t], flo=FT - 1)
            if t + 1 < KT:
                eops.append(emit_p4_early(1, t + 1, fhi=FT - 1))

    nc.compile()
    return nc


_NC_CACHE = None


def _get_nc():
    global _NC_CACHE
    if _NC_CACHE is None:
        _NC_CACHE = build_nc()
    return _NC_CACHE


def kernel(x, qkv_w, qkv_b, proj_w, proj_b, w, t_h=8, t_w=8, s_h=24, s_w=24):
    x = np.asarray(x, dtype=np.float32)
    qkv_w = np.asarray(qkv_w, dtype=np.float32)
    qkv_b = np.asarray(qkv_b, dtype=np.float32)
    proj_w = np.asarray(proj_w, dtype=np.float32)
    proj_b = np.asarray(proj_b, dtype=np.float32)
    w = np.asarray(w, dtype=np.float32)

    we = np.exp(w - w.max())
    ws = we / we.sum()
    w0, w1 = float(ws[0]), float(ws[1])

    qkv_w2 = qkv_w.copy()
    qkv_w2[:, 1536:] *= w0           # fold w0 into v columns

    auxb = np.zeros((128, 13), np.float32)
    for f in range(12):
        fcol = f * 128 if f < 6 else 768 + (f - 6) * 128
        auxb[:, f] = qkv_b[fcol:fcol + 128]
    auxb[:, 12] = math.sqrt(w1 / w0) * SCALE   # relu prescale

    vbb = np.tile((w0 * qkv_b[1536:2304])[None, :], (128, 1)).astype(np.float32)
    pbb = np.tile(proj_b[None, :], (128, 1)).astype(np.float32)

    common = {"qkv_w": qkv_w2.astype(np.float16),
              "proj_w": proj_w.astype(np.float16),
              "auxb": auxb, "vbb": vbb, "pbb": pbb}
    in_maps = []
    for c in range(NCORES):
        m = dict(common)
        m["xT"] = np.ascontiguousarray(
            x[c * BPC:(c + 1) * BPC].reshape(TOK, D).T).astype(np.float16)
        in_maps.append(m)

    nc = _get_nc()
    res = run_bass_kernel_spmd(nc, in_maps, core_ids=list(range(NCORES)))
    out = np.concatenate(
        [r["out"].reshape(BPC, N, D) for r in res.results], axis=0)
    return out.astype(np.float32)
